# revision 3
# baseline (speedup 1.0000x reference)
"""Trainium2 Bass kernel for nn_EntropyLoss (retrieval_knn) — v2.

Computes var([E(f1)-E(f0), E(f2)-E(f1)], ddof=1) where E(f) = log(1 +
sum r_ball) and r_ball[b,i] is the K-th NN distance (rank 52 incl self)
among the C=512 channel vectors (dim 4096) of sample b.

v2 strategy (vs v1's 7-round max8/match_replace selection, which was
DVE-bound at ~250us):

PE (symmetric Gram, ~2/3 the matmul work):
  m = G + bias[c] accumulated in PSUM; only column blocks c >= I are
  computed directly for row-block I; columns c < I are filled by PE
  transposes (is_transpose matmul) of earlier blocks' m tiles, with the
  spurious per-partition bias term removed during the Act PSUM->SBUF
  copy (bias AP = -bias[row]). Bias row is a K=2 matmul: fp16(2048 -
  sq/2) + const 512 row (keeps fp16 rounding small while making all
  off-diag m positive, which the masked selection needs).

Selection (threshold + 2-3 max8 rounds instead of 7 rounds):
  rank-52-largest of each m row == K-th NN radius. Host sends per-row
  t0 ~ mu + z*sigma (Gaussian tail estimate of the rank-44 value) and a
  secant slope u. Device: c0 = #{m > t0} (DVE tensor_scalar is_gt with
  fused accum); t1 = t0 + (c0-44)*u (Act tiny ops); c1 = #{m > t1}
  (Act Sign pass with fused accum, runs parallel to selection); km =
  (m <= t1)*m (DVE scalar_tensor_tensor -- killed elements become 0 <
  all kept, since m > 0); 2-3 max8/match_replace rounds give the top
  W kept values; the (52-c1)-th (clamped to [1,W]) is extracted with a
  per-row tensor_mask_reduce window. Rows where c1 falls outside
  [52-W, 51] (rare, calibrated) pick a neighboring order statistic --
  sub-0.1 r error on a handful of rows.

Host: d2 = sq_i + 5120 - 2*m_sel, r = sqrt(max(d2,0)), log/var tail in
fp64.
"""
import sys

for _p in ("/opt/trn_rl_repo", "/root/.axon_site/_ro/trn_rl_repo"):
    if _p not in sys.path:
        sys.path.insert(0, _p)

import numpy as np

from concourse import bacc, mybir, masks
from concourse.tile import TileContext
from concourse.bass_utils import run_bass_kernel_spmd

B, C, H, W_ = 16, 512, 64, 64
D = H * W_  # 4096
K = C // 10  # 51
RANK = K + 1  # 52: rank among descending m (incl diag)
N_CORES = 8
N_TENSORS = 3
UNITS = N_TENSORS * B  # 48
UPC = UNITS // N_CORES  # 6
KCHUNKS = D // 128  # 32
RBLK = C // 128  # 4
NBLK = UPC * RBLK  # 24
DMA_SPLIT = 4
BIAS_C = 512.0  # extra constant bias row (makes all m positive)

# --- calibrated constants (see calib.py; fit on real-data row statistics) ---
Z = 1.359114  # t0 = mu + Z*sig targets count ~C_TGT
KK = 79.6150  # u = sig / KK (secant slope)
SIG_SCALE = 1.176095
C_TGT = 44.0
SEL_W = 16  # extraction window; 8*ceil(W/8) max8 values kept
N_ROUNDS = (SEL_W + 7) // 8

TRACE = False
_LAST = {}

AF = mybir.ActivationFunctionType
ALU = mybir.AluOpType


def _build_program(repeat=1, loop_n=None):
    nc = bacc.Bacc("TRN2", target_bir_lowering=False, debug=False)

    xt_d = nc.dram_tensor(
        "xt", [UPC, 128, KCHUNKS * C], mybir.dt.float16, kind="ExternalInput"
    )
    sqn2_d = nc.dram_tensor(
        "sqn2", [2, UPC * C], mybir.dt.float16, kind="ExternalInput"
    )
    aux_names = ["nt0s", "u2s", "t212s", "nu2s", "nt212s", "corr"]
    aux_d = {
        n: nc.dram_tensor(n, [128, NBLK], mybir.dt.float32, kind="ExternalInput")
        for n in aux_names
    }
    msel_d = nc.dram_tensor(
        "msel", [128, NBLK], mybir.dt.float32, kind="ExternalOutput"
    )

    kper = KCHUNKS // DMA_SPLIT
    xt_view = xt_d.ap().rearrange("s p (d k c) -> s p d k c", d=DMA_SPLIT, k=kper)

    with TileContext(nc) as tc:
        with (
            tc.tile_pool(name="xpool", bufs=2 * DMA_SPLIT) as xpool,
            tc.tile_pool(name="consts", bufs=1) as consts,
            tc.tile_pool(name="mpool", bufs=RBLK * 2) as mpool,
            tc.tile_pool(name="kpool", bufs=2) as kpool,
            tc.tile_pool(name="spool", bufs=4) as spool,
            tc.tile_pool(name="vpool", bufs=4) as vpool,
            tc.tile_pool(name="cpool", bufs=32) as cpool,
            tc.tile_pool(name="gps", bufs=4, space="PSUM") as gps,
        ):
            ones2 = consts.tile([2, 128], mybir.dt.float16)
            nc.vector.memset(ones2, 1.0)
            ident = consts.tile([128, 128], mybir.dt.float32)
            masks.make_identity(nc, ident[:])

            def constcol(val):
                t = consts.tile([128, 1], mybir.dt.float32, tag=f"const_{val}")
                nc.vector.memset(t, float(val))
                return t

            iota16 = consts.tile([128, SEL_W], mybir.dt.float32, tag="iota16")
            nc.gpsimd.iota(
                iota16, pattern=[[1, SEL_W]], base=0, channel_multiplier=0,
                allow_small_or_imprecise_dtypes=True,
            )
            msel_sb = consts.tile([128, NBLK], mybir.dt.float32)
            sqn2_sb = consts.tile([2, UPC * C], mybir.dt.float16)
            nc.sync.dma_start(out=sqn2_sb, in_=sqn2_d.ap())
            aux_sb = {}
            for n in aux_names:
                t = consts.tile([128, NBLK], mybir.dt.float32, tag=f"aux_{n}")
                nc.sync.dma_start(out=t, in_=aux_d[n].ap())
                aux_sb[n] = t

            def pipeline_body(_iv=None):
                for s in range(UPC):
                    xparts = []
                    for d in range(DMA_SPLIT):
                        xp = xpool.tile([128, kper, C], mybir.dt.float16, tag="xts")
                        nc.sync.dma_start(out=xp, in_=xt_view[s, :, d])
                        xparts.append(xp)

                    sqn2_s = sqn2_sb[:, s * C : (s + 1) * C]
                    m_tiles = []
                    for I in range(RBLK):
                        blk = s * RBLK + I
                        lo = 128 * I

                        def col(name):
                            return aux_sb[name][:, blk : blk + 1]

                        g_ps = gps.tile([128, C], mybir.dt.float32, tag="g")
                        nc.tensor.matmul(
                            out=g_ps, lhsT=ones2, rhs=sqn2_s, start=True, stop=False
                        )
                        for k in range(KCHUNKS):
                            xp = xparts[k // kper]
                            kk = k % kper
                            nc.tensor.matmul(
                                out=g_ps[:, lo:C],
                                lhsT=xp[:, kk, lo : lo + 128],
                                rhs=xp[:, kk, lo:C],
                                start=False,
                                stop=(I == 0 and k == KCHUNKS - 1),
                            )
                        for J in range(I):
                            nc.tensor.matmul(
                                out=g_ps[:, 128 * J : 128 * (J + 1)],
                                lhsT=m_tiles[J][:, lo : lo + 128],
                                rhs=ident,
                                is_transpose=True,
                                start=False,
                                stop=(J == I - 1),
                            )

                        m_t = mpool.tile([128, C], mybir.dt.float32, tag="m")
                        if I > 0:
                            nc.scalar.activation(
                                out=m_t[:, 0:lo],
                                in_=g_ps[:, 0:lo],
                                func=AF.Identity,
                                bias=col("corr"),
                                scale=1.0,
                            )
                        nc.scalar.copy(out=m_t[:, lo:C], in_=g_ps[:, lo:C])
                        m_tiles.append(m_t)

                        # s0 = sum sign(m - t0) = 2*c0 - 512  (Act accum)
                        s0 = cpool.tile([128, 1], mybir.dt.float32, tag="s0")
                        scr = spool.tile([128, C], mybir.dt.float32, tag="scr")
                        nc.scalar.activation(
                            out=scr, in_=m_t, func=AF.Sign,
                            bias=col("nt0s"), scale=1.0, accum_out=s0,
                        )
                        # t1 = t0 + (c0 - C_TGT)*u = s0*(u/2) + (t0 + (256-C_TGT)*u)
                        t1 = cpool.tile([128, 1], mybir.dt.float32, tag="t1")
                        nc.vector.tensor_scalar(
                            out=t1, in0=s0, scalar1=col("u2s"),
                            scalar2=col("t212s"), op0=ALU.mult, op1=ALU.add,
                        )
                        nt1 = cpool.tile([128, 1], mybir.dt.float32, tag="nt1")
                        nc.vector.tensor_scalar(
                            out=nt1, in0=s0, scalar1=col("nu2s"),
                            scalar2=col("nt212s"), op0=ALU.mult, op1=ALU.add,
                        )
                        # km = (m <= t1) * m   (killed -> 0 < all kept m)
                        km = kpool.tile([128, C], mybir.dt.float32, tag="km")
                        nc.vector.scalar_tensor_tensor(
                            out=km, in0=m_t, scalar=t1, in1=m_t,
                            op0=ALU.is_le, op1=ALU.mult,
                        )
                        v24 = vpool.tile([128, 8 * N_ROUNDS], mybir.dt.float32, tag="v")
                        for r in range(N_ROUNDS):
                            if r > 0:
                                nc.vector.match_replace(
                                    out=km,
                                    in_to_replace=v24[:, 8 * r - 8 : 8 * r],
                                    in_values=km,
                                    imm_value=-1e30,
                                )
                            nc.vector.max(out=v24[:, 8 * r : 8 * r + 8], in_=km)
                        # c1 = #{m > t1} via Sign accum: s1 = 2*c1 - 512
                        s1 = cpool.tile([128, 1], mybir.dt.float32, tag="s1")
                        scr2 = spool.tile([128, C], mybir.dt.float32, tag="scr2")
                        nc.scalar.activation(
                            out=scr2, in_=m_t, func=AF.Sign,
                            bias=nt1, scale=1.0, accum_out=s1,
                        )
                        # start = clamp(51 - c1, 0, W-1) = clamp(-0.5*s1 - 205, ...)
                        a_c = cpool.tile([128, 1], mybir.dt.float32, tag="a")
                        nc.vector.tensor_scalar(
                            out=a_c, in0=s1, scalar1=-0.5, scalar2=-205.0,
                            op0=ALU.mult, op1=ALU.add,
                        )
                        st_c = cpool.tile([128, 1], mybir.dt.float32, tag="st")
                        nc.vector.tensor_scalar(
                            out=st_c, in0=a_c, scalar1=0.0,
                            scalar2=float(SEL_W - 1), op0=ALU.max, op1=ALU.min,
                        )
                        # pick v16[p, st]: suffix mask (iota >= st) * v16, max
                        ind = vpool.tile([128, SEL_W], mybir.dt.float32, tag="ind")
                        nc.vector.tensor_scalar(
                            out=ind, in0=iota16, scalar1=st_c, scalar2=None,
                            op0=ALU.is_ge,
                        )
                        vm = vpool.tile([128, SEL_W], mybir.dt.float32, tag="vm")
                        nc.vector.tensor_tensor(
                            out=vm, in0=v24[:, 0:SEL_W], in1=ind, op=ALU.mult
                        )
                        nc.vector.reduce_max(
                            out=msel_sb[:, blk : blk + 1], in_=vm,
                            axis=mybir.AxisListType.X,
                        )

            if loop_n is not None:
                with tc.For_i(0, loop_n, 1) as _iv:
                    pipeline_body(_iv)
            else:
                for _rep in range(repeat):
                    pipeline_body()

            nc.sync.dma_start(out=msel_d.ap(), in_=msel_sb)

    nc.compile()
    return nc


_PROGRAM = None


def _core_layout(arr):
    """[U, C] row-major -> per-core [128, NBLK] (partition=row-in-block)."""
    return (
        arr.reshape(N_CORES, UPC, RBLK, 128).transpose(0, 3, 1, 2)
        .reshape(N_CORES, 128, NBLK)
    )


def kernel(feat0, feat1, feat2):
    global _PROGRAM
    feats = np.stack(
        [np.asarray(f).reshape(B, C, D) for f in (feat0, feat1, feat2)]
    ).reshape(UNITS, C, D)

    sq64 = np.einsum(
        "ucd,ucd->uc", feats, feats, dtype=np.float64, casting="safe"
    )
    sqn16 = (2048.0 - sq64 / 2.0).astype(np.float16)
    sqnT32 = sqn16.astype(np.float32) + np.float32(BIAS_C)  # total col bias

    x16 = feats.astype(np.float16)
    xt = np.ascontiguousarray(
        x16.transpose(0, 2, 1)
        .reshape(UNITS, KCHUNKS, 128, C)
        .transpose(0, 2, 1, 3)
        .reshape(UNITS, 128, KCHUNKS * C)
    )

    # host t0/u estimates (Gaussian tail of each m row)
    x32 = x16.astype(np.float32)
    S = x32.sum(axis=1)  # [U, D]
    dotS = np.einsum("ucd,ud->uc", x32, S)
    mu = (dotS - sq64) / (C - 1) + (
        sqnT32.sum(axis=1, keepdims=True) - sqnT32
    ) / (C - 1)
    sig = np.sqrt(sq64 + sqnT32.var(axis=1, keepdims=True)) * SIG_SCALE
    t0 = (mu + Z * sig).astype(np.float32)
    uu = (sig / KK).astype(np.float32)
    t212 = (t0 + (256.0 - C_TGT) * uu).astype(np.float32)
    u2 = (uu / 2.0).astype(np.float32)

    aux = {
        "nt0s": _core_layout(-t0),
        "u2s": _core_layout(u2),
        "t212s": _core_layout(t212),
        "nu2s": _core_layout(-u2),
        "nt212s": _core_layout(-t212),
        "corr": _core_layout(-sqnT32),
    }
    sqn2 = np.empty((N_CORES, 2, UPC * C), dtype=np.float16)
    for c in range(N_CORES):
        sqn2[c, 0] = sqn16[c * UPC : (c + 1) * UPC].reshape(-1)
        sqn2[c, 1] = np.float16(BIAS_C)

    if _PROGRAM is None:
        _PROGRAM = _build_program()
    nc = _PROGRAM
    in_maps = [
        {
            "xt": xt[c * UPC : (c + 1) * UPC],
            "sqn2": sqn2[c],
            **{n: aux[n][c] for n in aux},
        }
        for c in range(N_CORES)
    ]
    out = run_bass_kernel_spmd(
        nc, in_maps, core_ids=list(range(N_CORES)), trace=TRACE
    )
    _LAST.clear()
    _LAST["results"] = out

    m52 = np.empty((UNITS, C), dtype=np.float64)
    for c in range(N_CORES):
        sel = out.results[c]["msel"].reshape(128, UPC, RBLK)
        m52[c * UPC : (c + 1) * UPC] = sel.transpose(1, 2, 0).reshape(UPC, C)

    d2 = sq64 + 2.0 * (2048.0 + BIAS_C) - 2.0 * m52
    r = np.sqrt(np.clip(d2, 0.0, None))
    _LAST["r"] = r
    sums = r.reshape(N_TENSORS, B * C).sum(axis=1)
    e = np.log(sums + 1.0)
    deltas = np.array([e[1] - e[0], e[2] - e[1]])
    var = deltas.var(ddof=1)
    return np.asarray(var, dtype=np.float32)


# revision 4
# speedup vs baseline: 1.0217x; 1.0217x over previous
"""Trainium2 Bass kernel for nn_EntropyLoss (retrieval_knn) — v2.

Computes var([E(f1)-E(f0), E(f2)-E(f1)], ddof=1) where E(f) = log(1 +
sum r_ball) and r_ball[b,i] is the K-th NN distance (rank 52 incl self)
among the C=512 channel vectors (dim 4096) of sample b.

v2 strategy (vs v1's 7-round max8/match_replace selection, which was
DVE-bound at ~250us):

PE (symmetric Gram, ~2/3 the matmul work):
  m = G + bias[c] accumulated in PSUM; only column blocks c >= I are
  computed directly for row-block I; columns c < I are filled by PE
  transposes (is_transpose matmul) of earlier blocks' m tiles, with the
  spurious per-partition bias term removed during the Act PSUM->SBUF
  copy (bias AP = -bias[row]). Bias row is a K=2 matmul: fp16(2048 -
  sq/2) + const 512 row (keeps fp16 rounding small while making all
  off-diag m positive, which the masked selection needs).

Selection (threshold + 2-3 max8 rounds instead of 7 rounds):
  rank-52-largest of each m row == K-th NN radius. Host sends per-row
  t0 ~ mu + z*sigma (Gaussian tail estimate of the rank-44 value) and a
  secant slope u. Device: c0 = #{m > t0} (DVE tensor_scalar is_gt with
  fused accum); t1 = t0 + (c0-44)*u (Act tiny ops); c1 = #{m > t1}
  (Act Sign pass with fused accum, runs parallel to selection); km =
  (m <= t1)*m (DVE scalar_tensor_tensor -- killed elements become 0 <
  all kept, since m > 0); 2-3 max8/match_replace rounds give the top
  W kept values; the (52-c1)-th (clamped to [1,W]) is extracted with a
  per-row tensor_mask_reduce window. Rows where c1 falls outside
  [52-W, 51] (rare, calibrated) pick a neighboring order statistic --
  sub-0.1 r error on a handful of rows.

Host: d2 = sq_i + 5120 - 2*m_sel, r = sqrt(max(d2,0)), log/var tail in
fp64.

Measured on HW (device For_i loop slope, 8 cores in parallel):
~150-156 us steady-state per pipeline (baseline v1: 249 us), rel err
1.40e-3 (v1: 9.4e-4; tolerance 2e-2). Engine model: PE ~115 us
(symmetric Gram at the fp16 roofline: 10/16 sub-blocks direct + 6
transposes), DVE ~77 us, Act ~76 us, DMA 25.2 MB ~72 us.

HW gotchas found by probing (see probe2.py): tensor_scalar accum_out is
a silent no-op, tensor_tensor_reduce and tensor_mask_reduce crash the
device; Act Sign/Identity with bias/scale APs + accum work. Hence
counts run on Act (Sign+accum), t1/clamp as fused DVE tensor_scalar
two-op affines, and the windowed rank-extract is iota-mask + reduce_max.
"""
import sys

for _p in ("/opt/trn_rl_repo", "/root/.axon_site/_ro/trn_rl_repo"):
    if _p not in sys.path:
        sys.path.insert(0, _p)

import numpy as np

from concourse import bacc, mybir, masks
from concourse.tile import TileContext
from concourse.bass_utils import run_bass_kernel_spmd

B, C, H, W_ = 16, 512, 64, 64
D = H * W_  # 4096
K = C // 10  # 51
RANK = K + 1  # 52: rank among descending m (incl diag)
N_CORES = 8
N_TENSORS = 3
UNITS = N_TENSORS * B  # 48
UPC = UNITS // N_CORES  # 6
KCHUNKS = D // 128  # 32
RBLK = C // 128  # 4
NBLK = UPC * RBLK  # 24
DMA_SPLIT = 4
BIAS_C = 512.0  # extra constant bias row (makes all m positive)

# --- calibrated constants (see calib.py; fit on real-data row statistics) ---
Z = 1.359114  # t0 = mu + Z*sig targets count ~C_TGT
KK = 79.6150  # u = sig / KK (secant slope)
SIG_SCALE = 1.176095
C_TGT = 44.0
SEL_W = 16  # extraction window; 8*ceil(W/8) max8 values kept
N_ROUNDS = (SEL_W + 7) // 8

TRACE = False
_LAST = {}

AF = mybir.ActivationFunctionType
ALU = mybir.AluOpType


def _build_program(repeat=1, loop_n=None):
    nc = bacc.Bacc("TRN2", target_bir_lowering=False, debug=False)

    xt_d = nc.dram_tensor(
        "xt", [UPC, 128, KCHUNKS * C], mybir.dt.float16, kind="ExternalInput"
    )
    sqn2_d = nc.dram_tensor(
        "sqn2", [2, UPC * C], mybir.dt.float16, kind="ExternalInput"
    )
    aux_names = ["nt0s", "u2s", "t212s", "nu2s", "nt212s", "corr"]
    aux_d = {
        n: nc.dram_tensor(n, [128, NBLK], mybir.dt.float32, kind="ExternalInput")
        for n in aux_names
    }
    msel_d = nc.dram_tensor(
        "msel", [128, NBLK], mybir.dt.float32, kind="ExternalOutput"
    )

    kper = KCHUNKS // DMA_SPLIT
    xt_view = xt_d.ap().rearrange("s p (d k c) -> s p d k c", d=DMA_SPLIT, k=kper)

    with TileContext(nc) as tc:
        with (
            tc.tile_pool(name="xpool", bufs=2 * DMA_SPLIT) as xpool,
            tc.tile_pool(name="consts", bufs=1) as consts,
            tc.tile_pool(name="mpool", bufs=RBLK * 2) as mpool,
            tc.tile_pool(name="kpool", bufs=2) as kpool,
            tc.tile_pool(name="spool", bufs=4) as spool,
            tc.tile_pool(name="vpool", bufs=4) as vpool,
            tc.tile_pool(name="cpool", bufs=32) as cpool,
            tc.tile_pool(name="gps", bufs=4, space="PSUM") as gps,
        ):
            ones2 = consts.tile([2, 128], mybir.dt.float16)
            nc.vector.memset(ones2, 1.0)
            ident = consts.tile([128, 128], mybir.dt.float32)
            masks.make_identity(nc, ident[:])

            def constcol(val):
                t = consts.tile([128, 1], mybir.dt.float32, tag=f"const_{val}")
                nc.vector.memset(t, float(val))
                return t

            iota16 = consts.tile([128, SEL_W], mybir.dt.float32, tag="iota16")
            nc.gpsimd.iota(
                iota16, pattern=[[1, SEL_W]], base=0, channel_multiplier=0,
                allow_small_or_imprecise_dtypes=True,
            )
            msel_sb = consts.tile([128, NBLK], mybir.dt.float32)
            sqn2_sb = consts.tile([2, UPC * C], mybir.dt.float16)
            nc.sync.dma_start(out=sqn2_sb, in_=sqn2_d.ap())
            aux_sb = {}
            for n in aux_names:
                t = consts.tile([128, NBLK], mybir.dt.float32, tag=f"aux_{n}")
                nc.sync.dma_start(out=t, in_=aux_d[n].ap())
                aux_sb[n] = t

            def pipeline_body(_iv=None):
                for s in range(UPC):
                    xparts = []
                    for d in range(DMA_SPLIT):
                        xp = xpool.tile([128, kper, C], mybir.dt.float16, tag="xts")
                        nc.sync.dma_start(out=xp, in_=xt_view[s, :, d])
                        xparts.append(xp)

                    sqn2_s = sqn2_sb[:, s * C : (s + 1) * C]
                    m_tiles = []
                    for I in range(RBLK):
                        blk = s * RBLK + I
                        lo = 128 * I

                        def col(name):
                            return aux_sb[name][:, blk : blk + 1]

                        g_ps = gps.tile([128, C], mybir.dt.float32, tag="g")
                        nc.tensor.matmul(
                            out=g_ps, lhsT=ones2, rhs=sqn2_s, start=True, stop=False
                        )
                        for k in range(KCHUNKS):
                            xp = xparts[k // kper]
                            kk = k % kper
                            nc.tensor.matmul(
                                out=g_ps[:, lo:C],
                                lhsT=xp[:, kk, lo : lo + 128],
                                rhs=xp[:, kk, lo:C],
                                start=False,
                                stop=(I == 0 and k == KCHUNKS - 1),
                            )
                        for J in range(I):
                            nc.tensor.matmul(
                                out=g_ps[:, 128 * J : 128 * (J + 1)],
                                lhsT=m_tiles[J][:, lo : lo + 128],
                                rhs=ident,
                                is_transpose=True,
                                start=False,
                                stop=(J == I - 1),
                            )

                        m_t = mpool.tile([128, C], mybir.dt.float32, tag="m")
                        if I > 0:
                            nc.scalar.activation(
                                out=m_t[:, 0:lo],
                                in_=g_ps[:, 0:lo],
                                func=AF.Identity,
                                bias=col("corr"),
                                scale=1.0,
                            )
                        nc.scalar.copy(out=m_t[:, lo:C], in_=g_ps[:, lo:C])
                        m_tiles.append(m_t)

                        # s0 = sum sign(m - t0) = 2*c0 - 512  (Act accum)
                        s0 = cpool.tile([128, 1], mybir.dt.float32, tag="s0")
                        scr = spool.tile([128, C], mybir.dt.float32, tag="scr")
                        nc.scalar.activation(
                            out=scr, in_=m_t, func=AF.Sign,
                            bias=col("nt0s"), scale=1.0, accum_out=s0,
                        )
                        # t1 = t0 + (c0 - C_TGT)*u = s0*(u/2) + (t0 + (256-C_TGT)*u)
                        t1 = cpool.tile([128, 1], mybir.dt.float32, tag="t1")
                        nc.vector.tensor_scalar(
                            out=t1, in0=s0, scalar1=col("u2s"),
                            scalar2=col("t212s"), op0=ALU.mult, op1=ALU.add,
                        )
                        nt1 = cpool.tile([128, 1], mybir.dt.float32, tag="nt1")
                        nc.vector.tensor_scalar(
                            out=nt1, in0=s0, scalar1=col("nu2s"),
                            scalar2=col("nt212s"), op0=ALU.mult, op1=ALU.add,
                        )
                        # km = (m <= t1) * m   (killed -> 0 < all kept m)
                        km = kpool.tile([128, C], mybir.dt.float32, tag="km")
                        nc.vector.scalar_tensor_tensor(
                            out=km, in0=m_t, scalar=t1, in1=m_t,
                            op0=ALU.is_le, op1=ALU.mult,
                        )
                        v24 = vpool.tile([128, 8 * N_ROUNDS], mybir.dt.float32, tag="v")
                        for r in range(N_ROUNDS):
                            if r > 0:
                                nc.vector.match_replace(
                                    out=km,
                                    in_to_replace=v24[:, 8 * r - 8 : 8 * r],
                                    in_values=km,
                                    imm_value=-1e30,
                                )
                            nc.vector.max(out=v24[:, 8 * r : 8 * r + 8], in_=km)
                        # c1 = #{m > t1} via Sign accum: s1 = 2*c1 - 512
                        s1 = cpool.tile([128, 1], mybir.dt.float32, tag="s1")
                        scr2 = spool.tile([128, C], mybir.dt.float32, tag="scr2")
                        nc.scalar.activation(
                            out=scr2, in_=m_t, func=AF.Sign,
                            bias=nt1, scale=1.0, accum_out=s1,
                        )
                        # start = clamp(51 - c1, 0, W-1) = clamp(-0.5*s1 - 205, ...)
                        a_c = cpool.tile([128, 1], mybir.dt.float32, tag="a")
                        nc.vector.tensor_scalar(
                            out=a_c, in0=s1, scalar1=-0.5, scalar2=-205.0,
                            op0=ALU.mult, op1=ALU.add,
                        )
                        st_c = cpool.tile([128, 1], mybir.dt.float32, tag="st")
                        nc.vector.tensor_scalar(
                            out=st_c, in0=a_c, scalar1=0.0,
                            scalar2=float(SEL_W - 1), op0=ALU.max, op1=ALU.min,
                        )
                        # pick v16[p, st]: suffix mask (iota >= st) * v16, max
                        ind = vpool.tile([128, SEL_W], mybir.dt.float32, tag="ind")
                        nc.vector.tensor_scalar(
                            out=ind, in0=iota16, scalar1=st_c, scalar2=None,
                            op0=ALU.is_ge,
                        )
                        vm = vpool.tile([128, SEL_W], mybir.dt.float32, tag="vm")
                        nc.vector.tensor_tensor(
                            out=vm, in0=v24[:, 0:SEL_W], in1=ind, op=ALU.mult
                        )
                        nc.vector.reduce_max(
                            out=msel_sb[:, blk : blk + 1], in_=vm,
                            axis=mybir.AxisListType.X,
                        )

            if loop_n is not None:
                with tc.For_i(0, loop_n, 1) as _iv:
                    pipeline_body(_iv)
            else:
                for _rep in range(repeat):
                    pipeline_body()

            nc.sync.dma_start(out=msel_d.ap(), in_=msel_sb)

    nc.compile()
    return nc


_PROGRAM = None


def _core_layout(arr):
    """[U, C] row-major -> per-core [128, NBLK] (partition=row-in-block)."""
    return (
        arr.reshape(N_CORES, UPC, RBLK, 128).transpose(0, 3, 1, 2)
        .reshape(N_CORES, 128, NBLK)
    )


def kernel(feat0, feat1, feat2):
    global _PROGRAM
    feats = np.stack(
        [np.asarray(f).reshape(B, C, D) for f in (feat0, feat1, feat2)]
    ).reshape(UNITS, C, D)

    sq64 = np.einsum(
        "ucd,ucd->uc", feats, feats, dtype=np.float64, casting="safe"
    )
    sqn16 = (2048.0 - sq64 / 2.0).astype(np.float16)
    sqnT32 = sqn16.astype(np.float32) + np.float32(BIAS_C)  # total col bias

    x16 = feats.astype(np.float16)
    xt = np.ascontiguousarray(
        x16.transpose(0, 2, 1)
        .reshape(UNITS, KCHUNKS, 128, C)
        .transpose(0, 2, 1, 3)
        .reshape(UNITS, 128, KCHUNKS * C)
    )

    # host t0/u estimates (Gaussian tail of each m row)
    x32 = x16.astype(np.float32)
    S = x32.sum(axis=1)  # [U, D]
    dotS = np.einsum("ucd,ud->uc", x32, S)
    mu = (dotS - sq64) / (C - 1) + (
        sqnT32.sum(axis=1, keepdims=True) - sqnT32
    ) / (C - 1)
    sig = np.sqrt(sq64 + sqnT32.var(axis=1, keepdims=True)) * SIG_SCALE
    t0 = (mu + Z * sig).astype(np.float32)
    uu = (sig / KK).astype(np.float32)
    t212 = (t0 + (256.0 - C_TGT) * uu).astype(np.float32)
    u2 = (uu / 2.0).astype(np.float32)

    aux = {
        "nt0s": _core_layout(-t0),
        "u2s": _core_layout(u2),
        "t212s": _core_layout(t212),
        "nu2s": _core_layout(-u2),
        "nt212s": _core_layout(-t212),
        "corr": _core_layout(-sqnT32),
    }
    sqn2 = np.empty((N_CORES, 2, UPC * C), dtype=np.float16)
    for c in range(N_CORES):
        sqn2[c, 0] = sqn16[c * UPC : (c + 1) * UPC].reshape(-1)
        sqn2[c, 1] = np.float16(BIAS_C)

    if _PROGRAM is None:
        _PROGRAM = _build_program()
    nc = _PROGRAM
    in_maps = [
        {
            "xt": xt[c * UPC : (c + 1) * UPC],
            "sqn2": sqn2[c],
            **{n: aux[n][c] for n in aux},
        }
        for c in range(N_CORES)
    ]
    out = run_bass_kernel_spmd(
        nc, in_maps, core_ids=list(range(N_CORES)), trace=TRACE
    )
    _LAST.clear()
    _LAST["results"] = out

    m52 = np.empty((UNITS, C), dtype=np.float64)
    for c in range(N_CORES):
        sel = out.results[c]["msel"].reshape(128, UPC, RBLK)
        m52[c * UPC : (c + 1) * UPC] = sel.transpose(1, 2, 0).reshape(UPC, C)

    d2 = sq64 + 2.0 * (2048.0 + BIAS_C) - 2.0 * m52
    r = np.sqrt(np.clip(d2, 0.0, None))
    _LAST["r"] = r
    sums = r.reshape(N_TENSORS, B * C).sum(axis=1)
    e = np.log(sums + 1.0)
    deltas = np.array([e[1] - e[0], e[2] - e[1]])
    var = deltas.var(ddof=1)
    return np.asarray(var, dtype=np.float32)


# revision 6
# speedup vs baseline: 1.4880x; 1.4564x over previous
"""Trainium2 Bass kernel for nn_EntropyLoss (retrieval_knn) — v2.

Computes var([E(f1)-E(f0), E(f2)-E(f1)], ddof=1) where E(f) = log(1 +
sum r_ball) and r_ball[b,i] is the K-th NN distance (rank 52 incl self)
among the C=512 channel vectors (dim 4096) of sample b.

v2 strategy (vs v1's 7-round max8/match_replace selection, which was
DVE-bound at ~250us):

PE (symmetric Gram, ~2/3 the matmul work):
  m = G + bias[c] accumulated in PSUM; only column blocks c >= I are
  computed directly for row-block I; columns c < I are filled by PE
  transposes (is_transpose matmul) of earlier blocks' m tiles, with the
  spurious per-partition bias term removed during the Act PSUM->SBUF
  copy (bias AP = -bias[row]). Bias row is a K=2 matmul: fp16(2048 -
  sq/2) + const 512 row (keeps fp16 rounding small while making all
  off-diag m positive, which the masked selection needs).

Selection (threshold + 2-3 max8 rounds instead of 7 rounds):
  rank-52-largest of each m row == K-th NN radius. Host sends per-row
  t0 ~ mu + z*sigma (Gaussian tail estimate of the rank-44 value) and a
  secant slope u. Device: c0 = #{m > t0} (DVE tensor_scalar is_gt with
  fused accum); t1 = t0 + (c0-44)*u (Act tiny ops); c1 = #{m > t1}
  (Act Sign pass with fused accum, runs parallel to selection); km =
  (m <= t1)*m (DVE scalar_tensor_tensor -- killed elements become 0 <
  all kept, since m > 0); 2-3 max8/match_replace rounds give the top
  W kept values; the (52-c1)-th (clamped to [1,W]) is extracted with a
  per-row tensor_mask_reduce window. Rows where c1 falls outside
  [52-W, 51] (rare, calibrated) pick a neighboring order statistic --
  sub-0.1 r error on a handful of rows.

Host: d2 = sq_i + 5120 - 2*m_sel, r = sqrt(max(d2,0)), log/var tail in
fp64.

Measured on HW (device For_i loop slope, 8 cores in parallel):
~150-156 us steady-state per pipeline (baseline v1: 249 us), rel err
1.40e-3 (v1: 9.4e-4; tolerance 2e-2). Engine model: PE ~115 us
(symmetric Gram at the fp16 roofline: 10/16 sub-blocks direct + 6
transposes), DVE ~77 us, Act ~76 us, DMA 25.2 MB ~72 us.

HW gotchas found by probing (see probe2.py): tensor_scalar accum_out is
a silent no-op, tensor_tensor_reduce and tensor_mask_reduce crash the
device; Act Sign/Identity with bias/scale APs + accum work. Hence
counts run on Act (Sign+accum), t1/clamp as fused DVE tensor_scalar
two-op affines, and the windowed rank-extract is iota-mask + reduce_max.
"""
import sys

for _p in ("/opt/trn_rl_repo", "/root/.axon_site/_ro/trn_rl_repo"):
    if _p not in sys.path:
        sys.path.insert(0, _p)

import numpy as np

from concourse import bacc, mybir, masks
from concourse.tile import TileContext
from concourse.bass_utils import run_bass_kernel_spmd

B, C, H, W_ = 16, 512, 64, 64
D = H * W_  # 4096
K = C // 10  # 51
RANK = K + 1  # 52: rank among descending m (incl diag)
N_CORES = 8
N_TENSORS = 3
UNITS = N_TENSORS * B  # 48
UPC = UNITS // N_CORES  # 6
KCHUNKS = D // 128  # 32
RBLK = C // 128  # 4
NBLK = UPC * RBLK  # 24
DMA_SPLIT = 4
BIAS_C = 512.0  # extra constant bias row (makes all m positive)

# --- calibrated constants (see calib.py; fit on real-data row statistics) ---
Z = 1.359114  # t0 = mu + Z*sig targets count ~C_TGT
KK = 79.6150  # u = sig / KK (secant slope)
SIG_SCALE = 1.176095
C_TGT = 44.0
SEL_W = 16  # extraction window; 8*ceil(W/8) max8 values kept
N_ROUNDS = (SEL_W + 7) // 8

TRACE = False
_LAST = {}

AF = mybir.ActivationFunctionType
ALU = mybir.AluOpType


def _build_program(repeat=1, loop_n=None, ablate=()):
    """ablate: subset of {"mm","dve","cnt"} for timing attribution runs."""
    nc = bacc.Bacc("TRN2", target_bir_lowering=False, debug=False)

    xt_d = nc.dram_tensor(
        "xt", [UPC, 128, KCHUNKS * C], mybir.dt.float16, kind="ExternalInput"
    )
    sqn2_d = nc.dram_tensor(
        "sqn2", [2, UPC * C], mybir.dt.float16, kind="ExternalInput"
    )
    aux_names = ["nt0s", "u2s", "t212s", "nu2s", "nt212s", "corr"]
    aux_d = {
        n: nc.dram_tensor(n, [128, NBLK], mybir.dt.float32, kind="ExternalInput")
        for n in aux_names
    }
    msel_d = nc.dram_tensor(
        "msel", [128, NBLK], mybir.dt.float32, kind="ExternalOutput"
    )

    kper = KCHUNKS // DMA_SPLIT
    xt_view = xt_d.ap().rearrange("s p (d k c) -> s p d k c", d=DMA_SPLIT, k=kper)

    with TileContext(nc) as tc:
        with (
            tc.tile_pool(name="xpool", bufs=2 * DMA_SPLIT) as xpool,
            tc.tile_pool(name="consts", bufs=1) as consts,
            tc.tile_pool(name="mpool", bufs=RBLK * 2) as mpool,
            tc.tile_pool(name="kpool", bufs=2) as kpool,
            tc.tile_pool(name="spool", bufs=4) as spool,
            tc.tile_pool(name="vpool", bufs=4) as vpool,
            tc.tile_pool(name="cpool", bufs=32) as cpool,
            tc.tile_pool(name="gps", bufs=4, space="PSUM") as gps,
        ):
            ones2 = consts.tile([2, 128], mybir.dt.float16)
            nc.vector.memset(ones2, 1.0)
            ident = consts.tile([128, 128], mybir.dt.float32)
            masks.make_identity(nc, ident[:])

            def constcol(val):
                t = consts.tile([128, 1], mybir.dt.float32, tag=f"const_{val}")
                nc.vector.memset(t, float(val))
                return t

            iota16 = consts.tile([128, SEL_W], mybir.dt.float32, tag="iota16")
            nc.gpsimd.iota(
                iota16, pattern=[[1, SEL_W]], base=0, channel_multiplier=0,
                allow_small_or_imprecise_dtypes=True,
            )
            msel_sb = consts.tile([128, NBLK], mybir.dt.float32)
            sqn2_sb = consts.tile([2, UPC * C], mybir.dt.float16)
            nc.sync.dma_start(out=sqn2_sb, in_=sqn2_d.ap())
            aux_sb = {}
            for n in aux_names:
                t = consts.tile([128, NBLK], mybir.dt.float32, tag=f"aux_{n}")
                nc.sync.dma_start(out=t, in_=aux_d[n].ap())
                aux_sb[n] = t

            def pipeline_body(_iv=None):
                for s in range(UPC):
                    xparts = []
                    for d in range(DMA_SPLIT):
                        xp = xpool.tile([128, kper, C], mybir.dt.float16, tag="xts")
                        nc.sync.dma_start(out=xp, in_=xt_view[s, :, d])
                        xparts.append(xp)

                    sqn2_s = sqn2_sb[:, s * C : (s + 1) * C]
                    m_tiles = []
                    for I in range(RBLK):
                        blk = s * RBLK + I
                        lo = 128 * I

                        def col(name):
                            return aux_sb[name][:, blk : blk + 1]

                        g_ps = gps.tile([128, C], mybir.dt.float32, tag="g")
                        nc.tensor.matmul(
                            out=g_ps, lhsT=ones2, rhs=sqn2_s, start=True, stop=False
                        )
                        nkc = 1 if "mm" in ablate else KCHUNKS
                        for k in range(nkc):
                            xp = xparts[k // kper]
                            kk = k % kper
                            nc.tensor.matmul(
                                out=g_ps[:, lo:C],
                                lhsT=xp[:, kk, lo : lo + 128],
                                rhs=xp[:, kk, lo:C],
                                start=False,
                                stop=(I == 0 and k == nkc - 1),
                            )
                        for J in range(I):
                            nc.tensor.matmul(
                                out=g_ps[:, 128 * J : 128 * (J + 1)],
                                lhsT=m_tiles[J][:, lo : lo + 128],
                                rhs=ident,
                                is_transpose=True,
                                start=False,
                                stop=(J == I - 1),
                            )

                        m_t = mpool.tile([128, C], mybir.dt.float32, tag="m")
                        if I > 0:
                            nc.scalar.activation(
                                out=m_t[:, 0:lo],
                                in_=g_ps[:, 0:lo],
                                func=AF.Identity,
                                bias=col("corr"),
                                scale=1.0,
                            )
                        nc.scalar.copy(out=m_t[:, lo:C], in_=g_ps[:, lo:C])
                        m_tiles.append(m_t)

                        # s0 = sum sign(m - t0) = 2*c0 - 512  (Act accum)
                        s0 = cpool.tile([128, 1], mybir.dt.float32, tag="s0")
                        if "cnt" in ablate:
                            nc.vector.memset(s0, -424.0)
                        else:
                            scr = spool.tile([128, C], mybir.dt.float32, tag="scr")
                            nc.scalar.activation(
                                out=scr, in_=m_t, func=AF.Sign,
                                bias=col("nt0s"), scale=1.0, accum_out=s0,
                            )
                        # t1 = t0 + (c0 - C_TGT)*u = s0*(u/2) + (t0 + (256-C_TGT)*u)
                        t1 = cpool.tile([128, 1], mybir.dt.float32, tag="t1")
                        nc.vector.tensor_scalar(
                            out=t1, in0=s0, scalar1=col("u2s"),
                            scalar2=col("t212s"), op0=ALU.mult, op1=ALU.add,
                        )
                        nt1 = cpool.tile([128, 1], mybir.dt.float32, tag="nt1")
                        nc.vector.tensor_scalar(
                            out=nt1, in0=s0, scalar1=col("nu2s"),
                            scalar2=col("nt212s"), op0=ALU.mult, op1=ALU.add,
                        )
                        if "dve" in ablate:
                            nc.vector.memset(msel_sb[:, blk : blk + 1], 0.0)
                            continue
                        # km = (m <= t1) * m   (killed -> 0 < all kept m)
                        km = kpool.tile([128, C], mybir.dt.float32, tag="km")
                        nc.vector.scalar_tensor_tensor(
                            out=km, in0=m_t, scalar=t1, in1=m_t,
                            op0=ALU.is_le, op1=ALU.mult,
                        )
                        v24 = vpool.tile([128, 8 * N_ROUNDS], mybir.dt.float32, tag="v")
                        for r in range(N_ROUNDS):
                            if r > 0:
                                nc.vector.match_replace(
                                    out=km,
                                    in_to_replace=v24[:, 8 * r - 8 : 8 * r],
                                    in_values=km,
                                    imm_value=-1e30,
                                )
                            nc.vector.max(out=v24[:, 8 * r : 8 * r + 8], in_=km)
                        # c1 = #{m > t1} via Sign accum: s1 = 2*c1 - 512
                        s1 = cpool.tile([128, 1], mybir.dt.float32, tag="s1")
                        if "cnt" in ablate:
                            nc.vector.memset(s1, -424.0)
                        else:
                            scr2 = spool.tile([128, C], mybir.dt.float32, tag="scr2")
                            nc.scalar.activation(
                                out=scr2, in_=m_t, func=AF.Sign,
                                bias=nt1, scale=1.0, accum_out=s1,
                            )
                        # start = clamp(51 - c1, 0, W-1) = clamp(-0.5*s1 - 205, ...)
                        a_c = cpool.tile([128, 1], mybir.dt.float32, tag="a")
                        nc.vector.tensor_scalar(
                            out=a_c, in0=s1, scalar1=-0.5, scalar2=-205.0,
                            op0=ALU.mult, op1=ALU.add,
                        )
                        st_c = cpool.tile([128, 1], mybir.dt.float32, tag="st")
                        nc.vector.tensor_scalar(
                            out=st_c, in0=a_c, scalar1=0.0,
                            scalar2=float(SEL_W - 1), op0=ALU.max, op1=ALU.min,
                        )
                        # pick v16[p, st]: suffix mask (iota >= st) * v16, max
                        ind = vpool.tile([128, SEL_W], mybir.dt.float32, tag="ind")
                        nc.vector.tensor_scalar(
                            out=ind, in0=iota16, scalar1=st_c, scalar2=None,
                            op0=ALU.is_ge,
                        )
                        vm = vpool.tile([128, SEL_W], mybir.dt.float32, tag="vm")
                        nc.vector.tensor_tensor(
                            out=vm, in0=v24[:, 0:SEL_W], in1=ind, op=ALU.mult
                        )
                        nc.vector.reduce_max(
                            out=msel_sb[:, blk : blk + 1], in_=vm,
                            axis=mybir.AxisListType.X,
                        )

            if loop_n is not None:
                with tc.For_i(0, loop_n, 1) as _iv:
                    pipeline_body(_iv)
            else:
                for _rep in range(repeat):
                    pipeline_body()

            nc.sync.dma_start(out=msel_d.ap(), in_=msel_sb)

    nc.compile()
    return nc


_PROGRAM = None


def _core_layout(arr):
    """[U, C] row-major -> per-core [128, NBLK] (partition=row-in-block)."""
    return (
        arr.reshape(N_CORES, UPC, RBLK, 128).transpose(0, 3, 1, 2)
        .reshape(N_CORES, 128, NBLK)
    )


def kernel(feat0, feat1, feat2):
    global _PROGRAM
    feats = np.stack(
        [np.asarray(f).reshape(B, C, D) for f in (feat0, feat1, feat2)]
    ).reshape(UNITS, C, D)

    sq64 = np.einsum(
        "ucd,ucd->uc", feats, feats, dtype=np.float64, casting="safe"
    )
    sqn16 = (2048.0 - sq64 / 2.0).astype(np.float16)
    sqnT32 = sqn16.astype(np.float32) + np.float32(BIAS_C)  # total col bias

    x16 = feats.astype(np.float16)
    xt = np.ascontiguousarray(
        x16.transpose(0, 2, 1)
        .reshape(UNITS, KCHUNKS, 128, C)
        .transpose(0, 2, 1, 3)
        .reshape(UNITS, 128, KCHUNKS * C)
    )

    # host t0/u estimates (Gaussian tail of each m row)
    x32 = x16.astype(np.float32)
    S = x32.sum(axis=1)  # [U, D]
    dotS = np.einsum("ucd,ud->uc", x32, S)
    mu = (dotS - sq64) / (C - 1) + (
        sqnT32.sum(axis=1, keepdims=True) - sqnT32
    ) / (C - 1)
    sig = np.sqrt(sq64 + sqnT32.var(axis=1, keepdims=True)) * SIG_SCALE
    t0 = (mu + Z * sig).astype(np.float32)
    uu = (sig / KK).astype(np.float32)
    t212 = (t0 + (256.0 - C_TGT) * uu).astype(np.float32)
    u2 = (uu / 2.0).astype(np.float32)

    aux = {
        "nt0s": _core_layout(-t0),
        "u2s": _core_layout(u2),
        "t212s": _core_layout(t212),
        "nu2s": _core_layout(-u2),
        "nt212s": _core_layout(-t212),
        "corr": _core_layout(-sqnT32),
    }
    sqn2 = np.empty((N_CORES, 2, UPC * C), dtype=np.float16)
    for c in range(N_CORES):
        sqn2[c, 0] = sqn16[c * UPC : (c + 1) * UPC].reshape(-1)
        sqn2[c, 1] = np.float16(BIAS_C)

    if _PROGRAM is None:
        _PROGRAM = _build_program()
    nc = _PROGRAM
    in_maps = [
        {
            "xt": xt[c * UPC : (c + 1) * UPC],
            "sqn2": sqn2[c],
            **{n: aux[n][c] for n in aux},
        }
        for c in range(N_CORES)
    ]
    out = run_bass_kernel_spmd(
        nc, in_maps, core_ids=list(range(N_CORES)), trace=TRACE
    )
    _LAST.clear()
    _LAST["results"] = out

    m52 = np.empty((UNITS, C), dtype=np.float64)
    for c in range(N_CORES):
        sel = out.results[c]["msel"].reshape(128, UPC, RBLK)
        m52[c * UPC : (c + 1) * UPC] = sel.transpose(1, 2, 0).reshape(UPC, C)

    d2 = sq64 + 2.0 * (2048.0 + BIAS_C) - 2.0 * m52
    r = np.sqrt(np.clip(d2, 0.0, None))
    _LAST["r"] = r
    sums = r.reshape(N_TENSORS, B * C).sum(axis=1)
    e = np.log(sums + 1.0)
    deltas = np.array([e[1] - e[0], e[2] - e[1]])
    var = deltas.var(ddof=1)
    return np.asarray(var, dtype=np.float32)
